# revision 5
# baseline (speedup 1.0000x reference)
"""Trainium2 Bass kernel for a 2-layer MoE GPT (moe_routing).

Model: tok_emb lookup -> 2x [RMSNorm -> causal ALiBi attention -> RMSNorm ->
top-2-of-4 MoE (dense expert compute, sparse combine)] -> RMSNorm -> tied head.

Sharding over 8 NeuronCores:
- attention: head-parallel (2 of 16 heads per core), o_proj partial-summed
  via AllReduce.
- MoE: expert-parallel (expert e=c//2, hidden half c%2 per core), partial
  down_proj outputs AllReduced (routing weights folded in pre-reduce).
- LM head: vocab-parallel (4000 rows per core), concatenated on host.

Routing (top-2 expert selection + combine weights) is computed on the host
in fp32/fp64 - the router's rank-2/3 logit gap can be ~3e-4, far below
device activation-table noise, so on-device selection is not reproducible
against the fp32 reference. Heavy matmuls run bf16 x bf16 (full PE rate,
half the DMA of fp32); accumulation stays fp32 in PSUM.
"""

import sys

sys.path.insert(0, "/opt/trn_rl_repo")

import numpy as np

import concourse.bass as bass  # noqa: E402
import concourse.bacc as bacc  # noqa: E402
import concourse.tile as tile  # noqa: E402
from concourse import mybir  # noqa: E402
from concourse.bass_utils import run_bass_kernel_spmd  # noqa: E402

dt = mybir.dt
F32 = dt.float32
F32R = dt.float32r
BF16 = dt.bfloat16
F16 = dt.float16
AF = mybir.ActivationFunctionType
ALU = mybir.AluOpType

V, D, H, HD, L, E, HID, TOPK, B, T = 32000, 1024, 16, 64, 2, 4, 2048, 2, 1, 1024
EPS = 1e-8
NC_N = 8
VS = V // NC_N            # vocab shard per core (4000)
VC = 500                  # head matmul free-dim chunk (8 * 500 = 4000)
DC = D // 128             # 8 partition chunks of the model dim
HC = 1024 // 128          # 8 hidden chunks of the per-core expert slice
NQ = 2                    # T halves of 512 for matmul free dim
TQ = T // NQ              # 512
NEGF16 = -60000.0         # causal-mask bias (exp -> 0, fits fp16)

# (nq, kc) score tiles that are not fully causally masked, in emission order
ATT_TILES = [(nq, kc) for nq in range(NQ) for kc in range((nq + 1) * 4)]


def _ap(x):
    return x.ap() if hasattr(x, "ap") else x


def build_nc():
    nc = bacc.Bacc("TRN2", target_bir_lowering=False, debug=False,
                   num_devices=NC_N)

    # ---- DRAM I/O (per-core payloads; same shapes on every core) ----
    x0T = nc.dram_tensor("x0T", [D, T], F32, kind="ExternalInput")
    biasP = nc.dram_tensor("biasP", [2 * len(ATT_TILES), 128, TQ], F16,
                           kind="ExternalInput")
    qkvw = nc.dram_tensor("qkvw", [L, D, 384], BF16, kind="ExternalInput")
    ow = nc.dram_tensor("ow", [L, 128, D], BF16, kind="ExternalInput")
    gatew = nc.dram_tensor("gatew", [L, D, 1024], BF16, kind="ExternalInput")
    upw = nc.dram_tensor("upw", [L, D, 1024], BF16, kind="ExternalInput")
    downw = nc.dram_tensor("downw", [L, 1024, D], BF16, kind="ExternalInput")
    bcw = nc.dram_tensor("bcw", [L, 1, T], F32, kind="ExternalInput")
    headw = nc.dram_tensor("headw", [D, VS], BF16, kind="ExternalInput")
    cones = nc.dram_tensor("cones", [128, 128], F32, kind="ExternalInput")
    conesb = nc.dram_tensor("conesb", [128, 128], BF16, kind="ExternalInput")
    identb = nc.dram_tensor("identb", [128, 128], BF16, kind="ExternalInput")
    logits = nc.dram_tensor("logits", [T, VS], F16, kind="ExternalOutput")

    with tile.TileContext(nc) as tc, nc.allow_low_precision(
            reason="bf16 matmuls + fp32 PSUM accumulation; tolerance 2e-2"):
        _emit(nc, tc, x0T, biasP, qkvw, ow, gatew, upw, downw, bcw,
              headw, cones, conesb, identb, logits)
    nc.compile()
    return nc


def _emit(nc, tc, x0T, biasP, qkvw, ow, gatew, upw, downw, bcw,
          headw, cones, conesb, identb, logits):
    ctxpools = []
    cm_of = {}

    def pool(name, bufs=1, space="SBUF"):
        p = tc.tile_pool(name=name, bufs=bufs, space=space)
        pm = p.__enter__()
        ctxpools.append(pm)
        cm_of[id(pm)] = p
        return pm

    def close_pool(pm):
        cm_of[id(pm)].__exit__(None, None, None)
        ctxpools.remove(pm)

    outer = pool("outer", bufs=1)
    dram = pool("dram", bufs=1, space="DRAM")

    # constants
    ones_t = outer.tile([128, 128], F32R, name="ones", tag="ones", bufs=1)
    nc.sync.dma_start(out=ones_t[:], in_=_ap(cones)[:].bitcast(F32R))
    ones_b = outer.tile([128, 128], BF16, name="onesb", tag="onesb", bufs=1)
    nc.sync.dma_start(out=ones_b[:], in_=_ap(conesb)[:])
    id_b = outer.tile([128, 128], BF16, name="identb", tag="identb", bufs=1)
    nc.sync.dma_start(out=id_b[:], in_=_ap(identb)[:])

    # resident ALiBi+causal bias tiles (fp16), shared by both layers
    nbt = len(ATT_TILES)
    p_bias = pool("bias", bufs=1)
    btile = [p_bias.tile([128, TQ], F16, name=f"bias{i}", tag=f"bias{i}",
                         bufs=1) for i in range(2 * nbt)]
    for i in range(2 * nbt):
        nc.sync.dma_start(out=btile[i][:], in_=_ap(biasP)[i])

    # residual stream xT [D, T], 8 chunks of [128, T], fp32
    x = [outer.tile([128, T], F32, name=f"x{i}", tag=f"x{i}", bufs=1)
         for i in range(DC)]
    for i in range(DC):
        nc.sync.dma_start(out=x[i][:], in_=_ap(x0T)[i * 128:(i + 1) * 128, :])

    # normalized activations (bf16, matmul-ready)
    xn = [outer.tile([128, T], BF16, name=f"xn{i}", tag=f"xn{i}", bufs=1)
          for i in range(DC)]

    def rms_norm(tag):
        """xn[:] = bf16(x * rsqrt(mean_d(x^2) + EPS))."""
        p_sq = pool(f"sq_{tag}", bufs=1)
        p_ps = pool(f"nps_{tag}", bufs=1, space="PSUM")
        sq = [p_sq.tile([128, T], BF16, name=f"sq{i}", tag=f"sq{i}", bufs=1)
              for i in range(DC)]
        for i in range(DC):
            nc.vector.tensor_tensor(sq[i][:], x[i][:], x[i][:], ALU.mult)
        rstd = p_sq.tile([1, T], F32R, name="rstd", tag="rstd", bufs=1)
        rstdb = p_sq.tile([128, T], F32, name="rstdb", tag="rstdb", bufs=1)
        for nh in range(NQ):
            ssq = p_ps.tile([1, TQ], F32, name="ssq", tag="ssq", bufs=2)
            for i in range(DC):
                nc.tensor.matmul(ssq[:], ones_b[:, 0:1],
                                 sq[i][:, nh * TQ:(nh + 1) * TQ],
                                 start=(i == 0), stop=(i == DC - 1))
            t0 = p_sq.tile([1, TQ], F32, name="t0", tag="t0", bufs=2)
            nc.vector.tensor_scalar(t0[:], ssq[:], 1.0 / D, EPS,
                                    ALU.mult, ALU.add)
            t1 = p_sq.tile([1, TQ], F32, name="t1", tag="t1", bufs=2)
            nc.scalar.activation(t1[:], t0[:], AF.Sqrt)
            nc.vector.reciprocal(rstd[:, nh * TQ:(nh + 1) * TQ], t1[:])
            bps = p_ps.tile([128, TQ], F32, name="bps", tag="bps", bufs=2)
            nc.tensor.matmul(bps[:], ones_t[0:1, :].bitcast(F32R),
                             rstd[:, nh * TQ:(nh + 1) * TQ],
                             start=True, stop=True)
            nc.vector.tensor_copy(rstdb[:, nh * TQ:(nh + 1) * TQ], bps[:])
        for i in range(DC):
            nc.vector.tensor_tensor(xn[i][:], x[i][:], rstdb[:], ALU.mult)
        close_pool(p_ps)
        close_pool(p_sq)

    # ---------------- layers ----------------
    for l in range(L):
        # ---- attention ----
        rms_norm(f"a{l}")

        p_aw = pool(f"aw{l}", bufs=1)
        qkv_t = p_aw.tile([128, DC, 384], BF16, name="qkvw", tag="qkvw", bufs=1)
        nc.sync.dma_start(
            out=qkv_t[:],
            in_=_ap(qkvw)[l].rearrange("(a p) m -> p a m", p=128))
        ow_h = []
        for h in range(2):
            t = p_aw.tile([64, D], BF16, name=f"ow{h}", tag=f"ow{h}", bufs=1)
            nc.sync.dma_start(out=t[:],
                              in_=_ap(ow)[l][h * 64:(h + 1) * 64, :])
            ow_h.append(t)

        qkvT = p_aw.tile([128, 3, T], BF16, name="qkvT", tag="qkvT", bufs=1)
        p_qps = pool(f"qps{l}", bufs=1, space="PSUM")
        for m in range(3):
            for nh in range(NQ):
                ps = p_qps.tile([128, TQ], F32, name="qkv", tag="qkv", bufs=3)
                for i in range(DC):
                    nc.tensor.matmul(ps[:],
                                     qkv_t[:, i, m * 128:(m + 1) * 128],
                                     xn[i][:, nh * TQ:(nh + 1) * TQ],
                                     start=(i == 0), stop=(i == DC - 1))
                nc.vector.tensor_copy(
                    qkvT[:, m, nh * TQ:(nh + 1) * TQ], ps[:])
        qT = qkvT[:, 0, :]
        kT = qkvT[:, 1, :]
        vT = qkvT[:, 2, :]
        # v in natural [token, hd] layout via PE transposes; per-head tiles
        # with a ones column (65th) so the AV matmul also yields the softmax
        # denominator row.
        v_nat = [p_aw.tile([128, DC, 65], BF16, name=f"vnat{h}",
                           tag=f"vnat{h}", bufs=1) for h in range(2)]
        for tcn in range(DC):
            tp = p_qps.tile([128, 128], BF16, name="vt", tag="vt", bufs=2)
            nc.tensor.transpose(tp[:],
                                vT[:, tcn * 128:(tcn + 1) * 128],
                                id_b[:])
            for h in range(2):
                nc.vector.tensor_copy(v_nat[h][:, tcn, 0:64],
                                      tp[:, h * 64:(h + 1) * 64])
        for h in range(2):
            nc.vector.tensor_copy(v_nat[h][:, :, 64], ones_b[:, 0:DC])
        close_pool(p_qps)

        yTh = [p_aw.tile([64, T], BF16, name=f"yT{h}", tag=f"yT{h}", bufs=1)
               for h in range(2)]
        p_sps = pool(f"sps{l}", bufs=1, space="PSUM")
        p_sc = pool(f"sc{l}", bufs=1)
        for nq in range(NQ):
            kcs = [kc for (q, kc) in ATT_TILES if q == nq]
            yps = [p_sps.tile([65, TQ], F32, name=f"yt{h}", tag=f"yt{h}",
                              bufs=1) for h in range(2)]
            for ki, kc in enumerate(kcs):
                for h in range(2):
                    hp = h * 64
                    st = p_sps.tile([128, TQ], F32, name="st", tag="st", bufs=3)
                    nc.tensor.matmul(st[:],
                                     kT[hp:hp + 64, kc * 128:(kc + 1) * 128],
                                     qT[hp:hp + 64, nq * TQ:(nq + 1) * TQ],
                                     start=True, stop=True)
                    bti = (h * nbt + ATT_TILES.index((nq, kc)))
                    sc = p_sc.tile([128, TQ], F16, name="scs", tag="scs", bufs=4)
                    nc.vector.tensor_tensor(sc[:], st[:], btile[bti][:],
                                            ALU.add)
                    es = p_sc.tile([128, TQ], BF16, name="es", tag="es", bufs=4)
                    nc.scalar.activation(es[:], sc[:], AF.Exp)
                    nc.tensor.matmul(yps[h][:, :],
                                     v_nat[h][:, kc, :],
                                     es[:],
                                     start=(ki == 0), stop=(ki == len(kcs) - 1))
            # normalize: yTh = y_unnorm * (1/denom) broadcast over rows
            for h in range(2):
                rc = p_sc.tile([1, TQ], F32R, name="rc", tag="rc", bufs=2)
                nc.vector.reciprocal(rc[:], yps[h][64:65, :])
                rps = p_sps.tile([64, TQ], F32, name="rb", tag="rb", bufs=1)
                nc.tensor.matmul(rps[:], ones_t[0:1, 0:64].bitcast(F32R),
                                 rc[:], start=True, stop=True)
                rsb = p_sc.tile([64, TQ], F32, name="rsb", tag="rsb", bufs=2)
                nc.vector.tensor_copy(rsb[:], rps[:])
                nc.vector.tensor_tensor(
                    yTh[h][:, nq * TQ:(nq + 1) * TQ],
                    yps[h][0:64, :], rsb[:, :], ALU.mult)
        close_pool(p_sps)

        # o_proj partial [D, T] -> DRAM bounce -> AllReduce (bf16, split in
        # row-halves so the collective overlaps o_proj and residual adds)
        arin = [dram.tile([D // 2, T], BF16, name=f"arin_a{l}{s}",
                          tag=f"arin_a{l}{s}", bufs=1) for s in range(2)]
        arout = [dram.tile([D // 2, T], BF16, name=f"arout_a{l}{s}",
                           tag=f"arout_a{l}{s}", bufs=1, addr_space="Shared")
                 for s in range(2)]
        p_ops = pool(f"ops{l}", bufs=1, space="PSUM")
        for s in range(2):
            for i in range(s * 4, s * 4 + 4):
                for nh in range(NQ):
                    ps = p_ops.tile([128, TQ], F32, name="o", tag="o", bufs=4)
                    for h in range(2):
                        nc.tensor.matmul(ps[:],
                                         ow_h[h][:, i * 128:(i + 1) * 128],
                                         yTh[h][:, nh * TQ:(nh + 1) * TQ],
                                         start=(h == 0), stop=(h == 1))
                    st = p_sc.tile([128, TQ], BF16, name="ost", tag="ost",
                                   bufs=4)
                    nc.vector.tensor_copy(st[:], ps[:])
                    nc.sync.dma_start(
                        out=arin[s][(i - s * 4) * 128:(i - s * 4 + 1) * 128,
                                    nh * TQ:(nh + 1) * TQ],
                        in_=st[:])
            nc.gpsimd.collective_compute(
                "AllReduce", ALU.add, replica_groups=[list(range(NC_N))],
                ins=[arin[s].opt()], outs=[arout[s].opt()])
        close_pool(p_ops)
        for i in range(DC):
            s, io = i // 4, i % 4
            ar = p_sc.tile([128, T], BF16, name="arr", tag="arr", bufs=2)
            nc.sync.dma_start(out=ar[:],
                              in_=arout[s][io * 128:(io + 1) * 128, :])
            nc.vector.tensor_tensor(x[i][:], x[i][:], ar[:], ALU.add)
        for p in (p_sc, p_aw):
            close_pool(p)

        # ---- MoE (routing weights computed on host, shipped via bcw) ----
        rms_norm(f"f{l}")

        p_mw = pool(f"mw{l}", bufs=1)
        p_msc = pool(f"msc{l}", bufs=1)
        p_rps = pool(f"rps{l}", bufs=1, space="PSUM")
        # broadcast host combine weight row [1, T] -> [128, T] bf16
        bcrow = p_msc.tile([1, T], F32R, name="bcrow", tag="bcrow", bufs=1)
        nc.sync.dma_start(out=bcrow[:], in_=_ap(bcw)[l].bitcast(F32R))
        bc = p_msc.tile([128, T], BF16, name="bc", tag="bc", bufs=1)
        for nh in range(NQ):
            bp = p_rps.tile([128, TQ], F32, name="bcp", tag="bcp", bufs=2)
            nc.tensor.matmul(bp[:], ones_t[0:1, :].bitcast(F32R),
                             bcrow[:, nh * TQ:(nh + 1) * TQ],
                             start=True, stop=True)
            nc.vector.tensor_copy(bc[:, nh * TQ:(nh + 1) * TQ], bp[:])
        close_pool(p_rps)

        # expert slice: gate/up [D, 1024], down [1024, D]
        gate_t = p_mw.tile([128, DC, 1024], BF16, name="gate", tag="gate", bufs=1)
        nc.sync.dma_start(
            out=gate_t[:],
            in_=_ap(gatew)[l].rearrange("(a p) m -> p a m", p=128))
        up_t = p_mw.tile([128, DC, 1024], BF16, name="up", tag="up", bufs=1)
        nc.sync.dma_start(
            out=up_t[:],
            in_=_ap(upw)[l].rearrange("(a p) m -> p a m", p=128))
        down_t = p_mw.tile([128, HC, D], BF16, name="down", tag="down", bufs=1)
        nc.sync.dma_start(
            out=down_t[:],
            in_=_ap(downw)[l].rearrange("(a p) m -> p a m", p=128))

        # MoE partials AllReduced in bf16, split by token-halves (one per nh
        # pass) so the second half's matmuls overlap the first collective.
        arin2 = [dram.tile([D, TQ], BF16, name=f"arin_m{l}{s}",
                           tag=f"arin_m{l}{s}", bufs=1) for s in range(NQ)]
        arout2 = [dram.tile([D, TQ], BF16, name=f"arout_m{l}{s}",
                            tag=f"arout_m{l}{s}", bufs=1, addr_space="Shared")
                  for s in range(NQ)]
        p_mps = pool(f"mps{l}", bufs=1, space="PSUM")
        for nh in range(NQ):
            ts_ = slice(nh * TQ, (nh + 1) * TQ)
            gu = [p_msc.tile([128, TQ], BF16, name=f"gu{hc}", tag=f"gu{hc}",
                             bufs=1) for hc in range(HC)]
            for hc in range(HC):
                gps = p_mps.tile([128, TQ], F32, name="g", tag="g", bufs=2)
                for i in range(DC):
                    nc.tensor.matmul(gps[:],
                                     gate_t[:, i, hc * 128:(hc + 1) * 128],
                                     xn[i][:, ts_],
                                     start=(i == 0), stop=(i == DC - 1))
                gs = p_msc.tile([128, TQ], BF16, name="gs", tag="gs", bufs=2)
                nc.scalar.activation(gs[:], gps[:], AF.Silu)
                ups = p_mps.tile([128, TQ], F32, name="u", tag="u", bufs=2)
                for i in range(DC):
                    nc.tensor.matmul(ups[:],
                                     up_t[:, i, hc * 128:(hc + 1) * 128],
                                     xn[i][:, ts_],
                                     start=(i == 0), stop=(i == DC - 1))
                nc.vector.tensor_tensor(gu[hc][:], gs[:], ups[:], ALU.mult)
                nc.vector.tensor_tensor(gu[hc][:], gu[hc][:],
                                        bc[:, ts_], ALU.mult)
            for i in range(DC):
                dps = p_mps.tile([128, TQ], F32, name="d", tag="d", bufs=2)
                for hc in range(HC):
                    nc.tensor.matmul(dps[:],
                                     down_t[:, hc, i * 128:(i + 1) * 128],
                                     gu[hc][:],
                                     start=(hc == 0), stop=(hc == HC - 1))
                st = p_msc.tile([128, TQ], BF16, name="mo", tag="mo", bufs=4)
                nc.vector.tensor_copy(st[:], dps[:])
                nc.sync.dma_start(out=arin2[nh][i * 128:(i + 1) * 128, :],
                                  in_=st[:])
            nc.gpsimd.collective_compute(
                "AllReduce", ALU.add, replica_groups=[list(range(NC_N))],
                ins=[arin2[nh].opt()], outs=[arout2[nh].opt()])
        close_pool(p_mps)
        for nh in range(NQ):
            ts_ = slice(nh * TQ, (nh + 1) * TQ)
            for i in range(DC):
                ar = p_msc.tile([128, TQ], BF16, name="arr2", tag="arr2",
                                bufs=2)
                nc.sync.dma_start(out=ar[:],
                                  in_=arout2[nh][i * 128:(i + 1) * 128, :])
                nc.vector.tensor_tensor(x[i][:, ts_], x[i][:, ts_], ar[:],
                                        ALU.add)
        for p in (p_msc, p_mw):
            close_pool(p)

    # ---- final norm + vocab-sharded tied head ----
    rms_norm("h")
    p_hw = pool("hw", bufs=1)
    p_hps = pool("hps", bufs=1, space="PSUM")
    for vc in range(VS // VC):
        hw = p_hw.tile([128, DC, VC], BF16, name="hw", tag="hw", bufs=2)
        nc.sync.dma_start(
            out=hw[:],
            in_=_ap(headw).rearrange("(a p) m -> p a m", p=128)
            [:, :, vc * VC:(vc + 1) * VC])
        for tcn in range(DC):
            ps = p_hps.tile([128, VC], F32, name="h", tag="h", bufs=4)
            for i in range(DC):
                nc.tensor.matmul(ps[:],
                                 xn[i][:, tcn * 128:(tcn + 1) * 128],
                                 hw[:, i, :],
                                 start=(i == 0), stop=(i == DC - 1))
            lg = p_hw.tile([128, VC], F16, name="lg", tag="lg", bufs=4)
            nc.vector.tensor_copy(lg[:], ps[:])
            nc.sync.dma_start(
                out=_ap(logits)[tcn * 128:(tcn + 1) * 128,
                                vc * VC:(vc + 1) * VC],
                in_=lg[:])

    for pm in reversed(list(ctxpools)):
        close_pool(pm)


_NC_CACHE = None


def _get_nc():
    global _NC_CACHE
    if _NC_CACHE is None:
        _NC_CACHE = build_nc()
    return _NC_CACHE


def _host_routing(idx, tok_emb, attn_norm_w, q_w, q_b, kv_w, kv_b, o_w, o_b,
                  ffn_norm_w, router_w, gate_w, up_w, down_w):
    """Replicate the reference forward (numpy fp32 BLAS) far enough to get
    every router top-2 selection + combine weight.  Gaps between selected
    and rejected experts are >= 2.8e-4 for fp32-scale noise (~1e-6), so the
    selection is reproducible.  Returns combine [L, T, E] float32."""
    f32 = np.float32
    x = tok_emb[idx[0]].astype(f32)                       # [T, D]
    slopes = (np.arange(1, H + 1, dtype=f32) / H)
    pos = np.arange(T, dtype=f32)
    dposq = pos[None, :] - pos[:, None]                   # [q, k] = k - q
    causal = dposq > 0                                    # k > q masked
    combine = np.zeros((L, T, E), f32)

    def rms(v, w):
        return (v / np.sqrt(np.mean(v * v, axis=-1, keepdims=True) + EPS)
                * w).astype(f32)

    for l in range(L):
        h = rms(x, attn_norm_w[l])
        q = (h @ q_w[l] + q_b[l]).reshape(T, H, HD)
        kv = (h @ kv_w[l] + kv_b[l]).reshape(T, 2, H, HD)
        k, v = kv[:, 0], kv[:, 1]
        y = np.empty((T, H, HD), f32)
        for hh in range(H):
            att = (q[:, hh] @ k[:, hh].T) * (1.0 / np.sqrt(HD))
            att += slopes[hh] * dposq
            att[causal] = -np.inf
            att -= att.max(axis=1, keepdims=True)
            np.exp(att, out=att)
            att /= att.sum(axis=1, keepdims=True)
            y[:, hh] = att @ v[:, hh]
        x = x + y.reshape(T, D) @ o_w[l] + o_b[l]

        hm = rms(x, ffn_norm_w[l])
        rlog = hm @ router_w[l]                           # [T, E] fp32
        sel = np.argsort(-rlog, axis=1, kind="stable")[:, :TOPK]
        vals = np.take_along_axis(rlog, sel, axis=1).astype(np.float64)
        w = np.exp(vals - vals.max(axis=1, keepdims=True))
        w /= w.sum(axis=1, keepdims=True)
        np.put_along_axis(combine[l], sel, w.astype(f32), axis=1)

        if l < L - 1:
            # sparse top-2 MoE to advance x (only needed to route layer l+1)
            moe = np.zeros((T, D), f32)
            scale = f32(1.0 / np.sqrt(l + 1))
            for e in range(E):
                tok = np.nonzero(combine[l, :, e])[0]
                if len(tok) == 0:
                    continue
                he = hm[tok]
                g = he @ gate_w[l, e]
                g = g / (1.0 + np.exp(-g)) * (he @ up_w[l, e])
                moe[tok] += (combine[l, tok, e:e + 1] * scale) * \
                    (g @ down_w[l, e])
            x = x + moe
    return combine


def make_in_maps(idx, tok_emb, attn_norm_w, q_w, q_b, kv_w, kv_b, o_w, o_b,
                 ffn_norm_w, router_w, gate_w, up_w, down_w, lnf_w):
    """Host-side sharding: build the per-core input dicts."""
    import ml_dtypes
    bf16 = ml_dtypes.bfloat16
    f32 = np.float32
    idx = np.asarray(idx)
    tok_emb = np.asarray(tok_emb, f32)
    x0T = np.ascontiguousarray(tok_emb[idx[0]].T)  # [D, T]

    qw = np.asarray(q_w, f32).reshape(L, D, H, HD)
    kvw = np.asarray(kv_w, f32).reshape(L, D, 2, H, HD)
    owf = np.asarray(o_w, f32).reshape(L, H, HD, D)
    anw = np.asarray(attn_norm_w, f32)
    fnw = np.asarray(ffn_norm_w, f32)
    gw = np.asarray(gate_w, f32)
    uw = np.asarray(up_w, f32)
    dw = np.asarray(down_w, f32)
    lnf = np.asarray(lnf_w, f32)

    combine = _host_routing(
        idx, tok_emb, anw, np.asarray(q_w, f32), np.asarray(q_b, f32),
        np.asarray(kv_w, f32), np.asarray(kv_b, f32), np.asarray(o_w, f32),
        np.asarray(o_b, f32), fnw, np.asarray(router_w, f32), gw, uw, dw)

    cones = np.ones((128, 128), f32)
    conesb = np.ones((128, 128), bf16)
    identb = np.eye(128, dtype=f32).astype(bf16)

    in_maps = []
    for c in range(NC_N):
        h0 = 2 * c
        e_core, hh = c // 2, c % 2
        # attention bias tiles (alibi + causal), valid tiles only, fp16
        nbt = len(ATT_TILES)
        biasP = np.empty((2 * nbt, 128, TQ), np.float16)
        for hi in range(2):
            slope = (h0 + hi + 1) / H
            for ti, (nq, kc) in enumerate(ATT_TILES):
                k = kc * 128 + np.arange(128, dtype=f32)[:, None]
                q = (nq * TQ + np.arange(TQ, dtype=f32))[None, :]
                b = slope * (k - q)
                b[k > q] = NEGF16
                biasP[hi * nbt + ti] = b.astype(np.float16)
        # qkv weights: attn_norm folded in, q scaled by 1/sqrt(HD)
        qkvw = np.empty((L, D, 384), f32)
        for l in range(L):
            sc = anw[l][:, None]
            qkvw[l, :, 0:128] = (
                qw[l][:, h0:h0 + 2].reshape(D, 128) * sc / np.sqrt(HD))
            qkvw[l, :, 128:256] = kvw[l][:, 0, h0:h0 + 2].reshape(D, 128) * sc
            qkvw[l, :, 256:384] = kvw[l][:, 1, h0:h0 + 2].reshape(D, 128) * sc
        ow_c = np.ascontiguousarray(owf[:, h0:h0 + 2].reshape(L, 128, D))
        gatew = np.ascontiguousarray(
            gw[:, e_core, :, hh * 1024:(hh + 1) * 1024] * fnw[:, :, None])
        upw = np.ascontiguousarray(
            uw[:, e_core, :, hh * 1024:(hh + 1) * 1024] * fnw[:, :, None])
        downw = np.ascontiguousarray(dw[:, e_core, hh * 1024:(hh + 1) * 1024])
        # per-token combine weight for this core's expert, depth-scaled
        bcw = np.empty((L, 1, T), f32)
        for l in range(L):
            bcw[l, 0] = combine[l, :, e_core] / np.sqrt(l + 1)
        headw = np.ascontiguousarray(
            (tok_emb[c * VS:(c + 1) * VS] * lnf[None, :]).T)
        in_maps.append(dict(
            x0T=x0T, biasP=biasP, qkvw=qkvw.astype(bf16),
            ow=ow_c.astype(bf16), gatew=gatew.astype(bf16),
            upw=upw.astype(bf16), downw=downw.astype(bf16), bcw=bcw,
            headw=headw.astype(bf16), cones=cones, conesb=conesb,
            identb=identb))
    return in_maps


def kernel(**inputs):
    nc = _get_nc()
    in_maps = make_in_maps(**inputs)
    res = run_bass_kernel_spmd(nc, in_maps, list(range(NC_N)))
    logits = np.concatenate(
        [res.results[c]["logits"].astype(np.float32) for c in range(NC_N)],
        axis=1)
    return logits.reshape(B, T, V)


# revision 8
# speedup vs baseline: 1.1326x; 1.1326x over previous
"""Trainium2 Bass kernel for a 2-layer MoE GPT (moe_routing).

Model: tok_emb lookup -> 2x [RMSNorm -> causal ALiBi attention -> RMSNorm ->
top-2-of-4 MoE (dense expert compute, sparse combine)] -> RMSNorm -> tied head.

Sharding over 8 NeuronCores:
- attention: head-parallel (2 of 16 heads per core), o_proj partial-summed
  via AllReduce.
- MoE: expert-parallel (expert e=c//2, hidden half c%2 per core), partial
  down_proj outputs AllReduced (routing weights folded in pre-reduce).
- LM head: vocab-parallel (4000 rows per core), concatenated on host.

Routing (top-2 expert selection + combine weights) is computed on the host
in fp32/fp64 - the router's rank-2/3 logit gap can be ~3e-4, far below
device activation-table noise, so on-device selection is not reproducible
against the fp32 reference. Heavy matmuls run bf16 x bf16 (full PE rate,
half the DMA of fp32); accumulation stays fp32 in PSUM.
"""

import sys

sys.path.insert(0, "/opt/trn_rl_repo")

import numpy as np

import concourse.bass as bass  # noqa: E402
import concourse.bacc as bacc  # noqa: E402
import concourse.tile as tile  # noqa: E402
from concourse import mybir  # noqa: E402
from concourse.bass_utils import run_bass_kernel_spmd  # noqa: E402

dt = mybir.dt
F32 = dt.float32
F32R = dt.float32r
BF16 = dt.bfloat16
F16 = dt.float16
AF = mybir.ActivationFunctionType
ALU = mybir.AluOpType

V, D, H, HD, L, E, HID, TOPK, B, T = 32000, 1024, 16, 64, 2, 4, 2048, 2, 1, 1024
EPS = 1e-8
NC_N = 8
VS = V // NC_N            # vocab shard per core (4000)
VC = 500                  # head matmul free-dim chunk (8 * 500 = 4000)
DC = D // 128             # 8 partition chunks of the model dim
HC = 1024 // 128          # 8 hidden chunks of the per-core expert slice
NQ = 2                    # T halves of 512 for matmul free dim
TQ = T // NQ              # 512
NEGF16 = -60000.0         # causal-mask bias (exp -> 0, fits fp16)

# (nq, kc) score tiles that are not fully causally masked, in emission order
ATT_TILES = [(nq, kc) for nq in range(NQ) for kc in range((nq + 1) * 4)]


def _ap(x):
    return x.ap() if hasattr(x, "ap") else x


def build_nc():
    nc = bacc.Bacc("TRN2", target_bir_lowering=False, debug=False,
                   num_devices=NC_N)

    # ---- DRAM I/O (per-core payloads; same shapes on every core) ----
    x0T = nc.dram_tensor("x0T", [D, T], F32, kind="ExternalInput")
    biasP = nc.dram_tensor("biasP", [2 * len(ATT_TILES), 128, TQ], F16,
                           kind="ExternalInput")
    qkvw = nc.dram_tensor("qkvw", [L, D, 384], BF16, kind="ExternalInput")
    ow = nc.dram_tensor("ow", [L, 128, D], BF16, kind="ExternalInput")
    gatew = nc.dram_tensor("gatew", [L, D, 1024], BF16, kind="ExternalInput")
    upw = nc.dram_tensor("upw", [L, D, 1024], BF16, kind="ExternalInput")
    downw = nc.dram_tensor("downw", [L, 1024, D], BF16, kind="ExternalInput")
    bcw = nc.dram_tensor("bcw", [L, 1, T], F32, kind="ExternalInput")
    headw = nc.dram_tensor("headw", [D, VS], BF16, kind="ExternalInput")
    cones = nc.dram_tensor("cones", [128, 128], F32, kind="ExternalInput")
    conesb = nc.dram_tensor("conesb", [128, 128], BF16, kind="ExternalInput")
    identb = nc.dram_tensor("identb", [128, 128], BF16, kind="ExternalInput")
    logits = nc.dram_tensor("logits", [T, VS], F16, kind="ExternalOutput")

    with tile.TileContext(nc) as tc, nc.allow_low_precision(
            reason="bf16 matmuls + fp32 PSUM accumulation; tolerance 2e-2"):
        _emit(nc, tc, x0T, biasP, qkvw, ow, gatew, upw, downw, bcw,
              headw, cones, conesb, identb, logits)
    nc.compile()
    return nc


def _emit(nc, tc, x0T, biasP, qkvw, ow, gatew, upw, downw, bcw,
          headw, cones, conesb, identb, logits):
    ctxpools = []
    cm_of = {}

    def pool(name, bufs=1, space="SBUF"):
        p = tc.tile_pool(name=name, bufs=bufs, space=space)
        pm = p.__enter__()
        ctxpools.append(pm)
        cm_of[id(pm)] = p
        return pm

    def close_pool(pm):
        cm_of[id(pm)].__exit__(None, None, None)
        ctxpools.remove(pm)

    outer = pool("outer", bufs=1)
    dram = pool("dram", bufs=1, space="DRAM")

    # constants
    ones_t = outer.tile([128, 128], F32R, name="ones", tag="ones", bufs=1)
    nc.sync.dma_start(out=ones_t[:], in_=_ap(cones)[:].bitcast(F32R))
    ones_b = outer.tile([128, 128], BF16, name="onesb", tag="onesb", bufs=1)
    nc.sync.dma_start(out=ones_b[:], in_=_ap(conesb)[:])
    id_b = outer.tile([128, 128], BF16, name="identb", tag="identb", bufs=1)
    nc.sync.dma_start(out=id_b[:], in_=_ap(identb)[:])

    # resident ALiBi+causal bias tiles (fp16), shared by both layers
    nbt = len(ATT_TILES)
    p_bias = pool("bias", bufs=1)
    btile = [p_bias.tile([128, TQ], F16, name=f"bias{i}", tag=f"bias{i}",
                         bufs=1) for i in range(2 * nbt)]
    for i in range(2 * nbt):
        nc.sync.dma_start(out=btile[i][:], in_=_ap(biasP)[i])

    # residual stream xT [D, T], 8 chunks of [128, T], fp32
    x = [outer.tile([128, T], F32, name=f"x{i}", tag=f"x{i}", bufs=1)
         for i in range(DC)]
    for i in range(DC):
        nc.sync.dma_start(out=x[i][:], in_=_ap(x0T)[i * 128:(i + 1) * 128, :])

    # normalized activations (bf16, matmul-ready)
    xn = [outer.tile([128, T], BF16, name=f"xn{i}", tag=f"xn{i}", bufs=1)
          for i in range(DC)]

    def rms_norm(tag):
        """xn[:] = bf16(x * rsqrt(mean_d(x^2) + EPS))."""
        p_sq = pool(f"sq_{tag}", bufs=1)
        p_ps = pool(f"nps_{tag}", bufs=1, space="PSUM")
        sq = [p_sq.tile([128, T], BF16, name=f"sq{i}", tag=f"sq{i}", bufs=1)
              for i in range(DC)]
        for i in range(DC):
            nc.vector.tensor_tensor(sq[i][:], x[i][:], x[i][:], ALU.mult)
        rstd = p_sq.tile([1, T], F32R, name="rstd", tag="rstd", bufs=1)
        rstdb = p_sq.tile([128, T], F32, name="rstdb", tag="rstdb", bufs=1)
        for nh in range(NQ):
            ssq = p_ps.tile([1, TQ], F32, name="ssq", tag="ssq", bufs=2)
            for i in range(DC):
                nc.tensor.matmul(ssq[:], ones_b[:, 0:1],
                                 sq[i][:, nh * TQ:(nh + 1) * TQ],
                                 start=(i == 0), stop=(i == DC - 1))
            t0 = p_sq.tile([1, TQ], F32, name="t0", tag="t0", bufs=2)
            nc.vector.tensor_scalar(t0[:], ssq[:], 1.0 / D, EPS,
                                    ALU.mult, ALU.add)
            t1 = p_sq.tile([1, TQ], F32, name="t1", tag="t1", bufs=2)
            nc.scalar.activation(t1[:], t0[:], AF.Sqrt)
            nc.vector.reciprocal(rstd[:, nh * TQ:(nh + 1) * TQ], t1[:])
            bps = p_ps.tile([128, TQ], F32, name="bps", tag="bps", bufs=2)
            nc.tensor.matmul(bps[:], ones_t[0:1, :].bitcast(F32R),
                             rstd[:, nh * TQ:(nh + 1) * TQ],
                             start=True, stop=True)
            nc.vector.tensor_copy(rstdb[:, nh * TQ:(nh + 1) * TQ], bps[:])
        for i in range(DC):
            nc.vector.tensor_tensor(xn[i][:], x[i][:], rstdb[:], ALU.mult)
        close_pool(p_ps)
        close_pool(p_sq)

    # ---------------- layers ----------------
    for l in range(L):
        # ---- attention ----
        rms_norm(f"a{l}")

        p_aw = pool(f"aw{l}", bufs=1)
        qkv_t = p_aw.tile([128, DC, 384], BF16, name="qkvw", tag="qkvw", bufs=1)
        nc.sync.dma_start(
            out=qkv_t[:],
            in_=_ap(qkvw)[l].rearrange("(a p) m -> p a m", p=128))
        ow_h = []
        for h in range(2):
            t = p_aw.tile([64, D], BF16, name=f"ow{h}", tag=f"ow{h}", bufs=1)
            nc.sync.dma_start(out=t[:],
                              in_=_ap(ow)[l][h * 64:(h + 1) * 64, :])
            ow_h.append(t)

        qkvT = p_aw.tile([128, 3, T], BF16, name="qkvT", tag="qkvT", bufs=1)
        p_qps = pool(f"qps{l}", bufs=1, space="PSUM")
        for m in range(3):
            for nh in range(NQ):
                ps = p_qps.tile([128, TQ], F32, name="qkv", tag="qkv", bufs=3)
                for i in range(DC):
                    nc.tensor.matmul(ps[:],
                                     qkv_t[:, i, m * 128:(m + 1) * 128],
                                     xn[i][:, nh * TQ:(nh + 1) * TQ],
                                     start=(i == 0), stop=(i == DC - 1))
                nc.vector.tensor_copy(
                    qkvT[:, m, nh * TQ:(nh + 1) * TQ], ps[:])
        qT = qkvT[:, 0, :]
        kT = qkvT[:, 1, :]
        vT = qkvT[:, 2, :]
        # v in natural [token, hd] layout via PE transposes; per-head tiles
        # with a ones column (65th) so the AV matmul also yields the softmax
        # denominator row.
        v_nat = [p_aw.tile([128, DC, 65], BF16, name=f"vnat{h}",
                           tag=f"vnat{h}", bufs=1) for h in range(2)]
        for tcn in range(DC):
            tp = p_qps.tile([128, 128], BF16, name="vt", tag="vt", bufs=2)
            nc.tensor.transpose(tp[:],
                                vT[:, tcn * 128:(tcn + 1) * 128],
                                id_b[:])
            for h in range(2):
                nc.vector.tensor_copy(v_nat[h][:, tcn, 0:64],
                                      tp[:, h * 64:(h + 1) * 64])
        for h in range(2):
            nc.vector.tensor_copy(v_nat[h][:, :, 64], ones_b[:, 0:DC])
        close_pool(p_qps)

        yTh = [p_aw.tile([64, T], BF16, name=f"yT{h}", tag=f"yT{h}", bufs=1)
               for h in range(2)]
        p_sps = pool(f"sps{l}", bufs=1, space="PSUM")
        p_sc = pool(f"sc{l}", bufs=1)
        for nq in range(NQ):
            kcs = [kc for (q, kc) in ATT_TILES if q == nq]
            yps = [p_sps.tile([65, TQ], F32, name=f"yt{h}", tag=f"yt{h}",
                              bufs=1) for h in range(2)]
            for ki, kc in enumerate(kcs):
                for h in range(2):
                    hp = h * 64
                    st = p_sps.tile([128, TQ], F32, name="st", tag="st", bufs=3)
                    nc.tensor.matmul(st[:],
                                     kT[hp:hp + 64, kc * 128:(kc + 1) * 128],
                                     qT[hp:hp + 64, nq * TQ:(nq + 1) * TQ],
                                     start=True, stop=True)
                    bti = (h * nbt + ATT_TILES.index((nq, kc)))
                    sc = p_sc.tile([128, TQ], F16, name="scs", tag="scs", bufs=4)
                    nc.vector.tensor_tensor(sc[:], st[:], btile[bti][:],
                                            ALU.add)
                    es = p_sc.tile([128, TQ], BF16, name="es", tag="es", bufs=4)
                    nc.scalar.activation(es[:], sc[:], AF.Exp)
                    nc.tensor.matmul(yps[h][:, :],
                                     v_nat[h][:, kc, :],
                                     es[:],
                                     start=(ki == 0), stop=(ki == len(kcs) - 1))
            # normalize: yTh = y_unnorm * (1/denom) broadcast over rows
            for h in range(2):
                rc = p_sc.tile([1, TQ], F32R, name="rc", tag="rc", bufs=2)
                nc.vector.reciprocal(rc[:], yps[h][64:65, :])
                rps = p_sps.tile([64, TQ], F32, name="rb", tag="rb", bufs=1)
                nc.tensor.matmul(rps[:], ones_t[0:1, 0:64].bitcast(F32R),
                                 rc[:], start=True, stop=True)
                rsb = p_sc.tile([64, TQ], F32, name="rsb", tag="rsb", bufs=2)
                nc.vector.tensor_copy(rsb[:], rps[:])
                nc.vector.tensor_tensor(
                    yTh[h][:, nq * TQ:(nq + 1) * TQ],
                    yps[h][0:64, :], rsb[:, :], ALU.mult)
        close_pool(p_sps)

        # o_proj partial [D, T] -> DRAM bounce -> one bf16 AllReduce
        # (collectives carry ~0.8ms fixed cost here - never split them)
        arin = dram.tile([D, T], BF16, name=f"arin_a{l}", tag=f"arin_a{l}",
                         bufs=1)
        arout = dram.tile([D, T], BF16, name=f"arout_a{l}", tag=f"arout_a{l}",
                          bufs=1, addr_space="Shared")
        p_ops = pool(f"ops{l}", bufs=1, space="PSUM")
        for i in range(DC):
            for nh in range(NQ):
                ps = p_ops.tile([128, TQ], F32, name="o", tag="o", bufs=4)
                for h in range(2):
                    nc.tensor.matmul(ps[:],
                                     ow_h[h][:, i * 128:(i + 1) * 128],
                                     yTh[h][:, nh * TQ:(nh + 1) * TQ],
                                     start=(h == 0), stop=(h == 1))
                st = p_sc.tile([128, TQ], BF16, name="ost", tag="ost", bufs=4)
                nc.vector.tensor_copy(st[:], ps[:])
                nc.sync.dma_start(
                    out=arin[i * 128:(i + 1) * 128, nh * TQ:(nh + 1) * TQ],
                    in_=st[:])
        close_pool(p_ops)
        nc.gpsimd.collective_compute(
            "AllReduce", ALU.add, replica_groups=[list(range(NC_N))],
            ins=[arin.opt()], outs=[arout.opt()])
        for i in range(DC):
            ar = p_sc.tile([128, T], BF16, name="arr", tag="arr", bufs=2)
            nc.sync.dma_start(out=ar[:], in_=arout[i * 128:(i + 1) * 128, :])
            nc.vector.tensor_tensor(x[i][:], x[i][:], ar[:], ALU.add)
        for p in (p_sc, p_aw):
            close_pool(p)

        # ---- MoE (routing weights computed on host, shipped via bcw) ----
        rms_norm(f"f{l}")

        p_mw = pool(f"mw{l}", bufs=1)
        p_msc = pool(f"msc{l}", bufs=1)
        p_rps = pool(f"rps{l}", bufs=1, space="PSUM")
        # broadcast host combine weight row [1, T] -> [128, T] bf16
        bcrow = p_msc.tile([1, T], F32R, name="bcrow", tag="bcrow", bufs=1)
        nc.sync.dma_start(out=bcrow[:], in_=_ap(bcw)[l].bitcast(F32R))
        bc = p_msc.tile([128, T], BF16, name="bc", tag="bc", bufs=1)
        for nh in range(NQ):
            bp = p_rps.tile([128, TQ], F32, name="bcp", tag="bcp", bufs=2)
            nc.tensor.matmul(bp[:], ones_t[0:1, :].bitcast(F32R),
                             bcrow[:, nh * TQ:(nh + 1) * TQ],
                             start=True, stop=True)
            nc.vector.tensor_copy(bc[:, nh * TQ:(nh + 1) * TQ], bp[:])
        close_pool(p_rps)

        # expert slice: gate/up [D, 1024], down [1024, D]
        gate_t = p_mw.tile([128, DC, 1024], BF16, name="gate", tag="gate", bufs=1)
        nc.sync.dma_start(
            out=gate_t[:],
            in_=_ap(gatew)[l].rearrange("(a p) m -> p a m", p=128))
        up_t = p_mw.tile([128, DC, 1024], BF16, name="up", tag="up", bufs=1)
        nc.sync.dma_start(
            out=up_t[:],
            in_=_ap(upw)[l].rearrange("(a p) m -> p a m", p=128))
        down_t = p_mw.tile([128, HC, D], BF16, name="down", tag="down", bufs=1)
        nc.sync.dma_start(
            out=down_t[:],
            in_=_ap(downw)[l].rearrange("(a p) m -> p a m", p=128))

        # MoE partials -> one bf16 AllReduce per layer
        arin2 = dram.tile([D, T], BF16, name=f"arin_m{l}", tag=f"arin_m{l}",
                          bufs=1)
        arout2 = dram.tile([D, T], BF16, name=f"arout_m{l}",
                           tag=f"arout_m{l}", bufs=1, addr_space="Shared")
        p_mps = pool(f"mps{l}", bufs=1, space="PSUM")
        for nh in range(NQ):
            ts_ = slice(nh * TQ, (nh + 1) * TQ)
            gu = [p_msc.tile([128, TQ], BF16, name=f"gu{hc}", tag=f"gu{hc}",
                             bufs=1) for hc in range(HC)]
            for hc in range(HC):
                gps = p_mps.tile([128, TQ], F32, name="g", tag="g", bufs=2)
                for i in range(DC):
                    nc.tensor.matmul(gps[:],
                                     gate_t[:, i, hc * 128:(hc + 1) * 128],
                                     xn[i][:, ts_],
                                     start=(i == 0), stop=(i == DC - 1))
                gs = p_msc.tile([128, TQ], BF16, name="gs", tag="gs", bufs=2)
                nc.scalar.activation(gs[:], gps[:], AF.Silu)
                ups = p_mps.tile([128, TQ], F32, name="u", tag="u", bufs=2)
                for i in range(DC):
                    nc.tensor.matmul(ups[:],
                                     up_t[:, i, hc * 128:(hc + 1) * 128],
                                     xn[i][:, ts_],
                                     start=(i == 0), stop=(i == DC - 1))
                nc.vector.tensor_tensor(gu[hc][:], gs[:], ups[:], ALU.mult)
                nc.vector.tensor_tensor(gu[hc][:], gu[hc][:],
                                        bc[:, ts_], ALU.mult)
            for i in range(DC):
                dps = p_mps.tile([128, TQ], F32, name="d", tag="d", bufs=2)
                for hc in range(HC):
                    nc.tensor.matmul(dps[:],
                                     down_t[:, hc, i * 128:(i + 1) * 128],
                                     gu[hc][:],
                                     start=(hc == 0), stop=(hc == HC - 1))
                st = p_msc.tile([128, TQ], BF16, name="mo", tag="mo", bufs=4)
                nc.vector.tensor_copy(st[:], dps[:])
                nc.sync.dma_start(out=arin2[i * 128:(i + 1) * 128, ts_],
                                  in_=st[:])
        close_pool(p_mps)
        nc.gpsimd.collective_compute(
            "AllReduce", ALU.add, replica_groups=[list(range(NC_N))],
            ins=[arin2.opt()], outs=[arout2.opt()])
        for i in range(DC):
            ar = p_msc.tile([128, T], BF16, name="arr2", tag="arr2", bufs=2)
            nc.sync.dma_start(out=ar[:], in_=arout2[i * 128:(i + 1) * 128, :])
            nc.vector.tensor_tensor(x[i][:], x[i][:], ar[:], ALU.add)
        for p in (p_msc, p_mw):
            close_pool(p)

    # ---- final norm + vocab-sharded tied head ----
    rms_norm("h")
    p_hw = pool("hw", bufs=1)
    p_hps = pool("hps", bufs=1, space="PSUM")
    for vc in range(VS // VC):
        hw = p_hw.tile([128, DC, VC], BF16, name="hw", tag="hw", bufs=2)
        nc.sync.dma_start(
            out=hw[:],
            in_=_ap(headw).rearrange("(a p) m -> p a m", p=128)
            [:, :, vc * VC:(vc + 1) * VC])
        for tcn in range(DC):
            ps = p_hps.tile([128, VC], F32, name="h", tag="h", bufs=4)
            for i in range(DC):
                nc.tensor.matmul(ps[:],
                                 xn[i][:, tcn * 128:(tcn + 1) * 128],
                                 hw[:, i, :],
                                 start=(i == 0), stop=(i == DC - 1))
            lg = p_hw.tile([128, VC], F16, name="lg", tag="lg", bufs=4)
            nc.vector.tensor_copy(lg[:], ps[:])
            nc.sync.dma_start(
                out=_ap(logits)[tcn * 128:(tcn + 1) * 128,
                                vc * VC:(vc + 1) * VC],
                in_=lg[:])

    for pm in reversed(list(ctxpools)):
        close_pool(pm)


_NC_CACHE = None


def _get_nc():
    global _NC_CACHE
    if _NC_CACHE is None:
        _NC_CACHE = build_nc()
    return _NC_CACHE


def _host_routing(idx, tok_emb, attn_norm_w, q_w, q_b, kv_w, kv_b, o_w, o_b,
                  ffn_norm_w, router_w, gate_w, up_w, down_w):
    """Replicate the reference forward (numpy fp32 BLAS) far enough to get
    every router top-2 selection + combine weight.  Gaps between selected
    and rejected experts are >= 2.8e-4 for fp32-scale noise (~1e-6), so the
    selection is reproducible.  Returns combine [L, T, E] float32."""
    f32 = np.float32
    x = tok_emb[idx[0]].astype(f32)                       # [T, D]
    slopes = (np.arange(1, H + 1, dtype=f32) / H)
    pos = np.arange(T, dtype=f32)
    dposq = pos[None, :] - pos[:, None]                   # [q, k] = k - q
    causal = dposq > 0                                    # k > q masked
    combine = np.zeros((L, T, E), f32)

    def rms(v, w):
        return (v / np.sqrt(np.mean(v * v, axis=-1, keepdims=True) + EPS)
                * w).astype(f32)

    for l in range(L):
        h = rms(x, attn_norm_w[l])
        q = (h @ q_w[l] + q_b[l]).reshape(T, H, HD)
        kv = (h @ kv_w[l] + kv_b[l]).reshape(T, 2, H, HD)
        k, v = kv[:, 0], kv[:, 1]
        y = np.empty((T, H, HD), f32)
        for hh in range(H):
            att = (q[:, hh] @ k[:, hh].T) * (1.0 / np.sqrt(HD))
            att += slopes[hh] * dposq
            att[causal] = -np.inf
            att -= att.max(axis=1, keepdims=True)
            np.exp(att, out=att)
            att /= att.sum(axis=1, keepdims=True)
            y[:, hh] = att @ v[:, hh]
        x = x + y.reshape(T, D) @ o_w[l] + o_b[l]

        hm = rms(x, ffn_norm_w[l])
        rlog = hm @ router_w[l]                           # [T, E] fp32
        sel = np.argsort(-rlog, axis=1, kind="stable")[:, :TOPK]
        vals = np.take_along_axis(rlog, sel, axis=1).astype(np.float64)
        w = np.exp(vals - vals.max(axis=1, keepdims=True))
        w /= w.sum(axis=1, keepdims=True)
        np.put_along_axis(combine[l], sel, w.astype(f32), axis=1)

        if l < L - 1:
            # sparse top-2 MoE to advance x (only needed to route layer l+1)
            moe = np.zeros((T, D), f32)
            scale = f32(1.0 / np.sqrt(l + 1))
            for e in range(E):
                tok = np.nonzero(combine[l, :, e])[0]
                if len(tok) == 0:
                    continue
                he = hm[tok]
                g = he @ gate_w[l, e]
                g = g / (1.0 + np.exp(-g)) * (he @ up_w[l, e])
                moe[tok] += (combine[l, tok, e:e + 1] * scale) * \
                    (g @ down_w[l, e])
            x = x + moe
    return combine


def make_in_maps(idx, tok_emb, attn_norm_w, q_w, q_b, kv_w, kv_b, o_w, o_b,
                 ffn_norm_w, router_w, gate_w, up_w, down_w, lnf_w):
    """Host-side sharding: build the per-core input dicts."""
    import ml_dtypes
    bf16 = ml_dtypes.bfloat16
    f32 = np.float32
    idx = np.asarray(idx)
    tok_emb = np.asarray(tok_emb, f32)
    x0T = np.ascontiguousarray(tok_emb[idx[0]].T)  # [D, T]

    qw = np.asarray(q_w, f32).reshape(L, D, H, HD)
    kvw = np.asarray(kv_w, f32).reshape(L, D, 2, H, HD)
    owf = np.asarray(o_w, f32).reshape(L, H, HD, D)
    anw = np.asarray(attn_norm_w, f32)
    fnw = np.asarray(ffn_norm_w, f32)
    gw = np.asarray(gate_w, f32)
    uw = np.asarray(up_w, f32)
    dw = np.asarray(down_w, f32)
    lnf = np.asarray(lnf_w, f32)

    combine = _host_routing(
        idx, tok_emb, anw, np.asarray(q_w, f32), np.asarray(q_b, f32),
        np.asarray(kv_w, f32), np.asarray(kv_b, f32), np.asarray(o_w, f32),
        np.asarray(o_b, f32), fnw, np.asarray(router_w, f32), gw, uw, dw)

    cones = np.ones((128, 128), f32)
    conesb = np.ones((128, 128), bf16)
    identb = np.eye(128, dtype=f32).astype(bf16)

    in_maps = []
    for c in range(NC_N):
        h0 = 2 * c
        e_core, hh = c // 2, c % 2
        # attention bias tiles (alibi + causal), valid tiles only, fp16
        nbt = len(ATT_TILES)
        biasP = np.empty((2 * nbt, 128, TQ), np.float16)
        for hi in range(2):
            slope = (h0 + hi + 1) / H
            for ti, (nq, kc) in enumerate(ATT_TILES):
                k = kc * 128 + np.arange(128, dtype=f32)[:, None]
                q = (nq * TQ + np.arange(TQ, dtype=f32))[None, :]
                b = slope * (k - q)
                b[k > q] = NEGF16
                biasP[hi * nbt + ti] = b.astype(np.float16)
        # qkv weights: attn_norm folded in, q scaled by 1/sqrt(HD)
        qkvw = np.empty((L, D, 384), f32)
        for l in range(L):
            sc = anw[l][:, None]
            qkvw[l, :, 0:128] = (
                qw[l][:, h0:h0 + 2].reshape(D, 128) * sc / np.sqrt(HD))
            qkvw[l, :, 128:256] = kvw[l][:, 0, h0:h0 + 2].reshape(D, 128) * sc
            qkvw[l, :, 256:384] = kvw[l][:, 1, h0:h0 + 2].reshape(D, 128) * sc
        ow_c = np.ascontiguousarray(owf[:, h0:h0 + 2].reshape(L, 128, D))
        gatew = np.ascontiguousarray(
            gw[:, e_core, :, hh * 1024:(hh + 1) * 1024] * fnw[:, :, None])
        upw = np.ascontiguousarray(
            uw[:, e_core, :, hh * 1024:(hh + 1) * 1024] * fnw[:, :, None])
        downw = np.ascontiguousarray(dw[:, e_core, hh * 1024:(hh + 1) * 1024])
        # per-token combine weight for this core's expert, depth-scaled
        bcw = np.empty((L, 1, T), f32)
        for l in range(L):
            bcw[l, 0] = combine[l, :, e_core] / np.sqrt(l + 1)
        headw = np.ascontiguousarray(
            (tok_emb[c * VS:(c + 1) * VS] * lnf[None, :]).T)
        in_maps.append(dict(
            x0T=x0T, biasP=biasP, qkvw=qkvw.astype(bf16),
            ow=ow_c.astype(bf16), gatew=gatew.astype(bf16),
            upw=upw.astype(bf16), downw=downw.astype(bf16), bcw=bcw,
            headw=headw.astype(bf16), cones=cones, conesb=conesb,
            identb=identb))
    return in_maps


def kernel(**inputs):
    nc = _get_nc()
    in_maps = make_in_maps(**inputs)
    res = run_bass_kernel_spmd(nc, in_maps, list(range(NC_N)))
    logits = np.concatenate(
        [res.results[c]["logits"].astype(np.float32) for c in range(NC_N)],
        axis=1)
    return logits.reshape(B, T, V)
